# revision 7
# baseline (speedup 1.0000x reference)
"""ConvNeXt block (depthwise 7x7 -> LN -> MLP(4C) w/ GELU -> layerscale+residual)
on 8 Trainium2 NeuronCores, data-parallel over batch (2 images/core).

Layout strategy: channels-on-partitions for conv+MLP (contraction on K),
depthwise conv as 49 PSUM-accumulated diagonal matmuls over a width-padded
image buffer; LN stats via broadcast ones-matmul; LN affine folded into w1;
branch in bf16 (layerscale gamma=1e-6 makes branch precision non-critical),
residual add in fp32 token layout.
"""
import numpy as np
import ml_dtypes

B, H, W, C = 16, 56, 56, 384
D4 = 4 * C
EPS = 1e-6
NCORES = 8
IPC = B // NCORES          # images per core = 2
T = H * W                  # 3136 tokens per image
WP = 62                    # padded width (3 + 56 + 3)
HP = H + 2                 # 1 spare row each side (AP under/overrun safety)
NT = 448                   # tokens per strip  (8 rows * 56)
NSTRIP = 7                 # strips per image
CCN = C // 128             # 3 channel chunks
DDN = D4 // 128            # 12 hidden chunks
BLK = 112                  # tokens per 2-row transpose block
NBLK = T // BLK            # 28 blocks per image

# tap order: dh=3 row first so the first matmul fully covers every strip
TAPS = [(3, dw) for dw in range(7)] + [
    (dh, dw) for dh in range(7) if dh != 3 for dw in range(7)
]

_CACHE = {}


def _split_multi_waits(nc, bass_rust, mybir):
    ctr = 0
    for fn in nc.m.functions:
        for bb in fn.blocks:
            new_list = None
            for ins in list(bb.instructions):
                si = ins.sync_info
                if si is None or len(si.on_wait) <= 1:
                    continue
                waits = list(si.on_wait)
                ins.sync_info = bass_rust.SyncInfo(
                    on_wait=[waits[-1]], on_update=list(si.on_update)
                )
                if new_list is None:
                    new_list = list(bb.instructions)
                pos = new_list.index(ins)
                for w in waits[:-1]:
                    ctr += 1
                    es = mybir.InstEventSemaphore(name=f"ESW-{ctr}", ins=[], outs=[])
                    es.engine = ins.engine
                    es.sync_info = bass_rust.SyncInfo(on_wait=[w], on_update=[])
                    new_list.insert(pos, es)
                    pos += 1
            if new_list is not None:
                bb.instructions = new_list


def _build():
    import bass_rust
    import concourse.bass as bass
    import concourse.mybir as mybir
    import concourse.tile as tile
    from concourse.vector_clock import ScopedClock

    # walrus here allows only one sync-wait per instruction; split the tile
    # tail-drain waits across extra drains
    def _drain_patch(self, tick_clock, wait_clock):
        nc = self.nc
        drain_inst = nc.sync.drain()
        wait_clock.add_sem_waits(
            drain_inst.ins, ScopedClock({None: tick_clock.global_clock})
        )
        si = drain_inst.ins.sync_info
        if si is not None and len(si.on_wait) > 1:
            waits = list(si.on_wait)
            drain_inst.ins.sync_info = bass_rust.SyncInfo(
                on_wait=[waits[0]], on_update=list(si.on_update)
            )
            for w in waits[1:]:
                n = nc.sync.drain()
                n.ins.sync_info = bass_rust.SyncInfo(on_wait=[w], on_update=[])
        nc.all_engine_barrier()
        popped = nc._tile_sem_poison_stack.pop()
        assert popped is self._sem_poison
        nc.clear_and_free_semaphores(list(self.sems.allocated().values()))
        nc.all_engine_barrier()

    tile.TileContext._drain_and_barrier = _drain_patch

    F32 = mybir.dt.float32
    BF16 = mybir.dt.bfloat16
    AF = mybir.ActivationFunctionType
    OP = mybir.AluOpType

    nc = bass.Bass()
    xd = nc.dram_tensor("x", [IPC * T, C], F32, kind="ExternalInput")
    ktd = nc.dram_tensor("ktap", [C, 49], F32, kind="ExternalInput")
    idb = nc.dram_tensor("idbf", [128, 128], BF16, kind="ExternalInput")
    w1d = nc.dram_tensor("w1b", [C, D4], BF16, kind="ExternalInput")
    b1d = nc.dram_tensor("b1f", [128, DDN], F32, kind="ExternalInput")
    w2d = nc.dram_tensor("w2b", [D4, C], BF16, kind="ExternalInput")
    gsd = nc.dram_tensor("gammas", [128, CCN], F32, kind="ExternalInput")
    gbd = nc.dram_tensor("gb2", [128, CCN], F32, kind="ExternalInput")
    dwd = nc.dram_tensor("dwb", [128, CCN], F32, kind="ExternalInput")
    od = nc.dram_tensor("out", [IPC * T, C], F32, kind="ExternalOutput")

    with tile.TileContext(nc) as tc:
        with (
            tc.tile_pool(name="const", bufs=1) as constp,
            tc.tile_pool(name="diagp", bufs=1) as diagp,
            tc.tile_pool(name="pads", bufs=3) as padp,
            tc.tile_pool(name="io", bufs=3) as iop,
            tc.tile_pool(name="ybuf", bufs=3) as yp,
            tc.tile_pool(name="ynbuf", bufs=3) as ynp,
            tc.tile_pool(name="tbuf", bufs=3) as tbp,
            tc.tile_pool(name="hbuf", bufs=2) as hp,
            tc.tile_pool(name="dve", bufs=2) as dvep,
            tc.tile_pool(name="cpsum", bufs=2, space="PSUM") as cps,
            tc.tile_pool(name="mpsum", bufs=1, space="PSUM") as mps,
            tc.tile_pool(name="tpsum", bufs=1, space="PSUM") as tps,
            tc.tile_pool(name="opsum", bufs=1, space="PSUM") as ops,
        ):
            # ---- constants ----
            idbf = constp.tile([128, 128], BF16, tag="idbf")
            nc.sync.dma_start(out=idbf[:], in_=idb[:])
            ktc = [constp.tile([128, 49], F32, tag=f"ktc{cc}", name=f"ktc{cc}") for cc in range(CCN)]
            for cc in range(CCN):
                nc.sync.dma_start(out=ktc[cc][:], in_=ktd[cc * 128:(cc + 1) * 128, :])
            w1s = [constp.tile([128, D4], BF16, tag=f"w1s{cc}", name=f"w1s{cc}") for cc in range(CCN)]
            for cc in range(CCN):
                nc.sync.dma_start(out=w1s[cc][:], in_=w1d[cc * 128:(cc + 1) * 128, :])
            w2s = [constp.tile([128, C], BF16, tag=f"w2s{dd}", name=f"w2s{dd}") for dd in range(DDN)]
            for dd in range(DDN):
                nc.sync.dma_start(out=w2s[dd][:], in_=w2d[dd * 128:(dd + 1) * 128, :])
            b1s = constp.tile([128, DDN], F32, tag="b1s")
            nc.sync.dma_start(out=b1s[:], in_=b1d[:])
            gss = constp.tile([128, CCN], F32, tag="gss")
            nc.sync.dma_start(out=gss[:], in_=gsd[:])
            gbs = constp.tile([128, CCN], F32, tag="gbs")
            nc.sync.dma_start(out=gbs[:], in_=gbd[:])
            dws = constp.tile([128, CCN], F32, tag="dws")
            nc.sync.dma_start(out=dws[:], in_=dwd[:])
            onesb = constp.tile([128, 128], BF16, tag="onesb")
            nc.vector.memset(onesb[:], 1.0)
            epst = constp.tile([128, 1], F32, tag="epst")
            nc.vector.memset(epst[:], EPS)

            # ---- diagonal tap matrices (bf16) ----
            diag = {}
            for cc in range(CCN):
                for j in range(49):
                    d = diagp.tile([128, 128], BF16, tag=f"dg{cc}_{j}", name=f"dg{cc}_{j}")
                    nc.vector.tensor_scalar_mul(d[:], idbf[:], ktc[cc][:, j:j + 1])
                    diag[(cc, j)] = d

            for img in range(IPC):
                base = img * T
                # ---- stage A: padded channel-major bf16 image ----
                pads = []
                for cc in range(CCN):
                    p = padp.tile([128, HP, WP], BF16, tag="padt", name=f"padt{cc}")
                    nc.vector.memset(p[:], 0.0)
                    pads.append(p)
                for blk in range(NBLK):
                    xb = iop.tile([BLK, C], F32, tag="xin")
                    nc.sync.dma_start(
                        out=xb[:], in_=xd[base + blk * BLK: base + (blk + 1) * BLK, :])
                    xbb = iop.tile([BLK, C], BF16, tag="xbf")
                    nc.scalar.copy(out=xbb[:], in_=xb[:])
                    for cc in range(CCN):
                        pt = tps.tile([128, BLK], BF16, tag="ptr")
                        nc.tensor.transpose(
                            pt[:], xbb[:, cc * 128:(cc + 1) * 128],
                            idbf[:BLK, :BLK])
                        dst = pads[cc][:, 1 + 2 * blk: 3 + 2 * blk, 3:59]
                        nc.vector.tensor_copy(
                            dst, pt[:].rearrange("p (h w) -> p h w", w=56))

                # ---- stage B: depthwise conv (49 diag matmuls / strip) ----
                ys = []
                for cc in range(CCN):
                    y = yp.tile([128, T], BF16, tag="yt", name=f"yt{cc}")
                    ys.append(y)
                for cc in range(CCN):
                    pfull = pads[cc][:]
                    for s in range(NSTRIP):
                        h0 = s * 8
                        ps = cps.tile([128, 8, WP], F32, tag="cps")
                        nmm = 0
                        for j, (dh, dw) in enumerate(TAPS):
                            lo = max(h0, 3 - dh)
                            hi = min(h0 + 8, 59 - dh, 56)
                            if hi <= lo:
                                continue
                            off = (1 + lo + dh - 3) * WP + (dw - 3)
                            rhs = bass.AP(
                                pfull.tensor,
                                pfull.offset + off,
                                [pfull.ap[0], [WP, hi - lo], [1, WP]],
                            )
                            nc.tensor.matmul(
                                ps[:, lo - h0: hi - h0, :],
                                diag[(cc, j)][:],
                                rhs,
                                start=(nmm == 0),
                                stop=(j == len(TAPS) - 1),
                            )
                            nmm += 1
                        ydst = ys[cc][:, h0 * 56:(h0 + 8) * 56].rearrange(
                            "p (h w) -> p h w", w=56)
                        nc.scalar.activation(
                            out=ydst, in_=ps[:, :, 3:59], func=AF.Identity,
                            bias=dws[:, cc:cc + 1], scale=1.0)

                # ---- stage C: LN stats + normalize (per strip) ----
                yns = []
                for cc in range(CCN):
                    yn = ynp.tile([128, T], BF16, tag="ynt", name=f"ynt{cc}")
                    yns.append(yn)
                for s in range(NSTRIP):
                    r0, r1 = s * NT, (s + 1) * NT
                    msum = mps.tile([128, NT], F32, tag="msum")
                    for cc in range(CCN):
                        nc.tensor.matmul(
                            msum[:], onesb[:], ys[cc][:, r0:r1],
                            start=(cc == 0), stop=(cc == CCN - 1))
                    m2sum = mps.tile([128, NT], F32, tag="m2sum")
                    for cc in range(CCN):
                        ysq = dvep.tile([128, NT], BF16, tag="ysq")
                        nc.scalar.square(ysq[:], ys[cc][:, r0:r1])
                        nc.tensor.matmul(
                            m2sum[:], onesb[:], ysq[:],
                            start=(cc == 0), stop=(cc == CCN - 1))
                    mu = dvep.tile([128, NT], F32, tag="mu")
                    nc.vector.tensor_scalar_mul(mu[:], msum[:], 1.0 / C)
                    mu2 = dvep.tile([128, NT], F32, tag="mu2")
                    nc.vector.tensor_mul(mu2[:], mu[:], mu[:])
                    var = dvep.tile([128, NT], F32, tag="var")
                    nc.vector.scalar_tensor_tensor(
                        out=var[:], in0=m2sum[:], scalar=1.0 / C, in1=mu2[:],
                        op0=OP.mult, op1=OP.subtract)
                    std = dvep.tile([128, NT], F32, tag="std")
                    nc.scalar.activation(
                        out=std[:], in_=var[:], func=AF.Sqrt,
                        bias=epst[:], scale=1.0)
                    rstd = dvep.tile([128, NT], F32, tag="rstd")
                    nc.vector.reciprocal(out=rstd[:], in_=std[:])
                    for cc in range(CCN):
                        ydm = dvep.tile([128, NT], F32, tag="ydm")
                        nc.vector.tensor_sub(ydm[:], ys[cc][:, r0:r1], mu[:])
                        nc.vector.tensor_mul(yns[cc][:, r0:r1], ydm[:], rstd[:])

                # ---- stage D: MLP ----
                tbs = []
                for cc in range(CCN):
                    tb = tbp.tile([128, T], BF16, tag="tbt", name=f"tbt{cc}")
                    tbs.append(tb)
                for s in range(NSTRIP):
                    r0, r1 = s * NT, (s + 1) * NT
                    ht = hp.tile([128, DDN, NT], BF16, tag="ht")
                    for dd in range(DDN):
                        ph = mps.tile([128, NT], F32, tag="mm", bufs=2, name="ph")
                        for cc in range(CCN):
                            nc.tensor.matmul(
                                ph[:], w1s[cc][:, dd * 128:(dd + 1) * 128],
                                yns[cc][:, r0:r1],
                                start=(cc == 0), stop=(cc == CCN - 1))
                        nc.scalar.activation(
                            out=ht[:, dd, :], in_=ph[:], func=AF.Gelu_apprx_tanh,
                            bias=b1s[:, dd:dd + 1], scale=1.0)
                    for cc in range(CCN):
                        py = mps.tile([128, NT], F32, tag="mm", bufs=2, name="py")
                        for dd in range(DDN):
                            nc.tensor.matmul(
                                py[:], w2s[dd][:, cc * 128:(cc + 1) * 128],
                                ht[:, dd, :],
                                start=(dd == 0), stop=(dd == DDN - 1))
                        nc.scalar.activation(
                            out=tbs[cc][:, r0:r1], in_=py[:], func=AF.Identity,
                            bias=gbs[:, cc:cc + 1], scale=gss[:, cc:cc + 1])

                # ---- stage E: transpose back + residual + store ----
                for blk in range(NBLK):
                    pt = ops.tile([BLK, C], BF16, tag="optr")
                    for cc in range(CCN):
                        nc.tensor.transpose(
                            pt[:, cc * 128:(cc + 1) * 128],
                            tbs[cc][:, blk * BLK:(blk + 1) * BLK], idbf[:])
                    xb2 = iop.tile([BLK, C], F32, tag="xin2")
                    nc.sync.dma_start(
                        out=xb2[:], in_=xd[base + blk * BLK: base + (blk + 1) * BLK, :])
                    ob = iop.tile([BLK, C], F32, tag="ob")
                    nc.vector.tensor_add(ob[:], xb2[:], pt[:])
                    nc.sync.dma_start(
                        out=od[base + blk * BLK: base + (blk + 1) * BLK, :], in_=ob[:])

    nc.finalize()
    _split_multi_waits(nc, bass_rust, mybir)
    return nc


def kernel(x, dw_kernel, dw_bias, ln_scale, ln_bias, w1, b1, w2, b2, gamma):
    from concourse.bass_utils import run_bass_kernel_spmd

    if "nc" not in _CACHE:
        _CACHE["nc"] = _build()
    nc = _CACHE["nc"]

    x = np.asarray(x, dtype=np.float32)
    bf = ml_dtypes.bfloat16
    k2 = np.asarray(dw_kernel, np.float32)[:, :, 0, :]          # [7,7,C]
    ktap = np.stack([k2[dh, dw] for (dh, dw) in TAPS], axis=1)  # [C,49]
    w1f = (np.asarray(ln_scale, np.float32)[:, None]
           * np.asarray(w1, np.float32)).astype(bf)             # [C,4C]
    b1f = (np.asarray(b1, np.float32)
           + np.asarray(ln_bias, np.float32) @ np.asarray(w1, np.float32))
    b1f = b1f.reshape(DDN, 128).T.copy()                        # [128,12]
    w2b = np.asarray(w2, np.float32).astype(bf)                 # [4C,C]
    gam = np.asarray(gamma, np.float32)
    gammas = gam.reshape(CCN, 128).T.copy()
    gb2 = (gam * np.asarray(b2, np.float32)).reshape(CCN, 128).T.copy()
    dwb = np.asarray(dw_bias, np.float32).reshape(CCN, 128).T.copy()
    idbf = np.eye(128, dtype=bf)

    shared = {
        "ktap": np.ascontiguousarray(ktap, np.float32),
        "idbf": idbf, "w1b": np.ascontiguousarray(w1f),
        "b1f": np.ascontiguousarray(b1f, np.float32),
        "w2b": np.ascontiguousarray(w2b),
        "gammas": np.ascontiguousarray(gammas, np.float32),
        "gb2": np.ascontiguousarray(gb2, np.float32),
        "dwb": np.ascontiguousarray(dwb, np.float32),
    }
    in_maps = []
    for c in range(NCORES):
        xs = x[c * IPC:(c + 1) * IPC].reshape(IPC * T, C)
        in_maps.append({"x": np.ascontiguousarray(xs), **shared})

    res = run_bass_kernel_spmd(nc, in_maps, core_ids=list(range(NCORES)))
    out = np.empty((B, H, W, C), dtype=np.float32)
    for c in range(NCORES):
        out[c * IPC:(c + 1) * IPC] = res.results[c]["out"].reshape(IPC, H, W, C)
    _CACHE["last"] = res
    return out
